# revision 14
# baseline (speedup 1.0000x reference)
"""Trainium2 Bass kernel for nn_NetAndTexture (point+mesh rasterized feature
gather + SH2 evaluation).

Strategy (8 NeuronCores, SPMD):
  - Pixel/data parallel: shard the [2,H,W] fragment maps along H (96 rows per
    core); replicate the 500000x72 feature table in each core's HBM.
  - Host-side index marshaling only: faces[p2f] lookup is folded into flat
    int32 gather indices (4 per pixel: point idx + 3 triangle verts), masks
    are extracted from index signs, ray dirs are pre-flipped/flattened.
  - All gather offsets are preloaded once into a persistent SBUF tile (host
    pre-permuted to the on-chip pixel layout) so each indirect gather carries
    at most one cross-engine sync wait (walrus limits DMA wait commands).
  - The per-pixel 9-term SH basis is computed once per core into a persistent
    SBUF tile [128, 36, 16, 9] (frames share it).
  - Device per superblock of 2048 pixels (128 partitions x K=16 columns):
      one indirect DMA gathers 4x72 floats per pixel ->  G[128,K,4,72]
      tmp = G * (per-pixel SH basis | bary-scaled basis)  (4 DVE mults)
      red = segmented reduce over the 9 SH coeffs        -> [128,K,4,8]
      out[0:8]  = red[v=0] * (mask1 * (1 - d/R^2))
      out[8:16] = (red[v=1]+red[v=2]+red[v=3]) * mask2
"""

import numpy as np

# problem constants (hardcoded; kernel.py must be self-contained)
H, W = 768, 768
NCORES = 8
HROWS = H // NCORES            # 96
NPIX_F = HROWS * W             # 73728 pixels per frame per core
NPIX = 2 * NPIX_F              # 147456 pixels per core
P_TBL, C = 500000, 72
PART = 128                     # SBUF partitions
K = 16                         # pixel columns per superblock
SB_PIX = PART * K              # 2048
NSB = NPIX // SB_PIX           # 72
NSB_F = NSB // 2               # 36 (basis shared between the 2 frames)

RADIUS = 0.006
R2 = RADIUS * RADIUS
SH_C0 = 0.28209479177387814
SH_C1 = 0.4886025119029199
SH_C2 = (1.0925484305920792, -1.0925484305920792, 0.31539156525252005,
         -1.0925484305920792, 0.5462742152960396)

_BUILT = {}


def _build(feat_dtype="float32", nsb=NSB, split_waits=True):
    from concourse import bass, mybir
    import concourse.tile as tile

    f32 = mybir.dt.float32
    i32 = mybir.dt.int32
    fdt = getattr(mybir.dt, feat_dtype)
    MUL = mybir.AluOpType.mult
    ADD = mybir.AluOpType.add
    SUB = mybir.AluOpType.subtract
    MAX = mybir.AluOpType.max

    nc = bass.Bass()
    features = nc.declare_dram_parameter("features", [P_TBL, C], fdt, isOutput=False)
    # host-pre-permuted: [p, s*(K*4) + c*4 + f] = field f of pixel s*2048+p*16+c
    aux_i = nc.declare_dram_parameter("aux_i", [PART, NSB * K * 4], i32,
                                      isOutput=False)
    aux_f = nc.declare_dram_parameter("aux_f", [NPIX, 6], f32, isOutput=False)
    # host-pre-permuted like aux_i (frame 0 only): [p, sf*(K*3) + c*3 + xyz]
    dirs = nc.declare_dram_parameter("dirs", [PART, NSB_F * K * 3], f32,
                                     isOutput=False)
    out_feat = nc.declare_dram_parameter("out_feat", [NPIX, 16], f32, isOutput=True)
    out_mask = nc.declare_dram_parameter("out_mask", [NPIX], f32, isOutput=True)

    with tile.TileContext(nc) as tc:
        with (
            tc.tile_pool(name="persist", bufs=1) as pp,
            tc.tile_pool(name="pf", bufs=3) as pf,
            tc.tile_pool(name="pg", bufs=3) as pg,
            tc.tile_pool(name="ps", bufs=2) as ps,
            tc.tile_pool(name="pt", bufs=2) as pt,
            tc.tile_pool(name="pr", bufs=2) as pr,
            tc.tile_pool(name="po", bufs=3) as po,
            tc.tile_pool(name="pw", bufs=2) as pw,
        ):
            # ---- persistent: all gather offsets ------------------------
            auxo = pp.tile([PART, NSB * K * 4], i32, tag="auxo")
            nc.sync.dma_start(out=auxo[:], in_=aux_i[:, :])

            # ---- pre-pass: per-pixel SH basis into persistent SBUF -----
            dt_ = pp.tile([PART, NSB_F, K, 3], f32, tag="dt")
            nc.sync.dma_start(
                out=dt_[:].rearrange("p s k c -> p (s k c)"),
                in_=dirs[:, :],
            )
            bt = pp.tile([PART, NSB_F, K, 9], f32, tag="bt")
            x = dt_[:, :, :, 0]
            y = dt_[:, :, :, 1]
            z = dt_[:, :, :, 2]
            sxx = pp.tile([PART, NSB_F, K], f32, tag="sxx")
            syy = pp.tile([PART, NSB_F, K], f32, tag="syy")
            szz = pp.tile([PART, NSB_F, K], f32, tag="szz")
            st = pp.tile([PART, NSB_F, K], f32, tag="st")
            st2 = pp.tile([PART, NSB_F, K], f32, tag="st2")

            nc.vector.memset(bt[:, :, :, 0], SH_C0)
            nc.vector.tensor_scalar_mul(bt[:, :, :, 1], y, -SH_C1)
            nc.vector.tensor_scalar_mul(bt[:, :, :, 2], z, SH_C1)
            nc.vector.tensor_scalar_mul(bt[:, :, :, 3], x, -SH_C1)
            nc.vector.tensor_tensor(out=st[:], in0=x, in1=y, op=MUL)
            nc.vector.tensor_scalar_mul(bt[:, :, :, 4], st[:], SH_C2[0])
            nc.vector.tensor_tensor(out=st2[:], in0=y, in1=z, op=MUL)
            nc.vector.tensor_scalar_mul(bt[:, :, :, 5], st2[:], SH_C2[1])
            nc.vector.tensor_tensor(out=sxx[:], in0=x, in1=x, op=MUL)
            nc.vector.tensor_tensor(out=syy[:], in0=y, in1=y, op=MUL)
            nc.vector.tensor_tensor(out=szz[:], in0=z, in1=z, op=MUL)
            nc.vector.tensor_tensor(out=st[:], in0=x, in1=z, op=MUL)
            nc.vector.tensor_scalar_mul(bt[:, :, :, 7], st[:], SH_C2[3])
            # col6 = C2_2 * (2 zz - xx - yy)
            nc.vector.tensor_tensor(out=st2[:], in0=sxx[:], in1=syy[:], op=ADD)
            nc.vector.tensor_scalar_mul(st2[:], st2[:], SH_C2[2])
            nc.vector.tensor_scalar_mul(st[:], szz[:], 2.0 * SH_C2[2])
            nc.vector.tensor_tensor(out=bt[:, :, :, 6], in0=st[:], in1=st2[:], op=SUB)
            # col8 = C2_4 * (xx - yy)
            nc.vector.tensor_tensor(out=st[:], in0=sxx[:], in1=syy[:], op=SUB)
            nc.vector.tensor_scalar_mul(bt[:, :, :, 8], st[:], SH_C2[4])

            # ---- main loop over superblocks ----------------------------
            for s in range(nsb):
                sf = s % NSB_F
                rs, re = s * SB_PIX, (s + 1) * SB_PIX
                basq = bt[:, sf, :, :]                       # [128, K, 9]

                auxf = pf.tile([PART, K, 6], f32, tag="auxf")
                nc.sync.dma_start(
                    out=auxf[:],
                    in_=aux_f[rs:re, :].rearrange("(p k) c -> p (k c)", p=PART),
                )

                # walrus honors ONE dynamic offset per partition per DMA, so
                # each (column, vertex) needs its own 128-row gather
                G = pg.tile([PART, K, 4, C], fdt, tag="G")
                for c in range(K):
                    for v in range(4):
                        nc.gpsimd.indirect_dma_start(
                            out=G[:, c, v, :],
                            out_offset=None,
                            in_=features[:, :],
                            in_offset=bass.IndirectOffsetOnAxis(
                                ap=auxo[:, s * K * 4 + c * 4 + v
                                        :s * K * 4 + c * 4 + v + 1],
                                axis=0,
                            ),
                        )

                # bary-scaled bases for the 3 triangle verts
                sbv = ps.tile([PART, K, 3, 9], f32, tag="sbv")
                for v in range(3):
                    nc.vector.tensor_tensor(
                        out=sbv[:, :, v, :],
                        in0=basq,
                        in1=auxf[:, :, 1 + v].to_broadcast([PART, K, 9]),
                        op=MUL,
                    )

                # wm1 = m1 * (1 - d / R^2)
                wm1 = pw.tile([PART, K], f32, tag="wm1")
                nc.vector.tensor_scalar(
                    out=wm1[:], in0=auxf[:, :, 0],
                    scalar1=-1.0 / R2, scalar2=1.0, op0=MUL, op1=ADD,
                )
                nc.vector.tensor_tensor(
                    out=wm1[:], in0=wm1[:], in1=auxf[:, :, 4], op=MUL,
                )

                tmpA = pt.tile([PART, K, 4, C], f32, tag="tmpA")
                g4 = lambda apx: apx.rearrange("p k (g n) -> p k g n", n=9)
                nc.vector.tensor_tensor(
                    out=g4(tmpA[:, :, 0, :]),
                    in0=g4(G[:, :, 0, :]),
                    in1=basq.rearrange("p k (o n) -> p k o n", o=1)
                            .broadcast_to([PART, K, 8, 9]),
                    op=MUL,
                )
                for v in range(3):
                    nc.vector.tensor_tensor(
                        out=g4(tmpA[:, :, v + 1, :]),
                        in0=g4(G[:, :, v + 1, :]),
                        in1=sbv[:, :, v, :]
                            .rearrange("p k (o n) -> p k o n", o=1)
                            .broadcast_to([PART, K, 8, 9]),
                        op=MUL,
                    )

                red9 = pr.tile([PART, K, 4, 8], f32, tag="red9")
                nc.vector.tensor_reduce(
                    out=red9[:].rearrange("p k v g -> p (k v g)"),
                    in_=tmpA[:].rearrange("p k v (g n) -> p (k v g) n", n=9),
                    axis=mybir.AxisListType.X,
                    op=ADD,
                )

                outb = po.tile([PART, K, 16], f32, tag="outb")
                nc.vector.tensor_tensor(
                    out=outb[:, :, 0:8],
                    in0=red9[:, :, 0, :],
                    in1=wm1[:].to_broadcast([PART, K, 8]),
                    op=MUL,
                )
                t2 = pw.tile([PART, K, 8], f32, tag="t2")
                nc.vector.tensor_tensor(
                    out=t2[:], in0=red9[:, :, 1, :], in1=red9[:, :, 2, :], op=ADD,
                )
                nc.vector.tensor_tensor(
                    out=t2[:], in0=t2[:], in1=red9[:, :, 3, :], op=ADD,
                )
                nc.vector.tensor_tensor(
                    out=outb[:, :, 8:16],
                    in0=t2[:],
                    in1=auxf[:, :, 5].to_broadcast([PART, K, 8]),
                    op=MUL,
                )

                mk = pw.tile([PART, K], f32, tag="mk")
                nc.vector.tensor_tensor(
                    out=mk[:], in0=auxf[:, :, 4], in1=auxf[:, :, 5], op=MAX,
                )

                nc.sync.dma_start(
                    out=out_feat[rs:re, :].rearrange("(p k) c -> p (k c)", p=PART),
                    in_=outb[:],
                )
                nc.sync.dma_start(
                    out=out_mask[rs:re].rearrange("(p k) -> p k", p=PART),
                    in_=mk[:],
                )

    if split_waits:
        _split_excess_waits(nc, mybir)
    return nc


def _split_excess_waits(nc, mybir, limit=1):
    """Walrus rejects instructions with too many sync-wait commands. Split
    excess waits off onto preceding same-engine NoOps (waits execute in
    order on the engine's sequencer, so semantics are preserved)."""
    for f in nc.m.functions:
        for bb in f.blocks:
            i = 0
            insts = bb.instructions
            while i < len(insts):
                inst = insts[i]
                si = inst.sync_info
                if si is not None and len(si.on_wait) > limit:
                    waits = list(si.on_wait)
                    extra, keep = waits[:-limit], waits[-limit:]
                    carriers = []
                    for j in range(0, len(extra), limit):
                        carriers.append(
                            mybir.InstNoOp(
                                name=f"{inst.name}-waitsplit-{j}",
                                engine=inst.engine,
                                ins=[],
                                outs=[],
                                sync_info=mybir.SyncInfo(
                                    on_wait=extra[j:j + limit], on_update=[]
                                ),
                            )
                        )
                    si.on_wait = keep
                    for c_off, c in enumerate(carriers):
                        insts.insert(i + c_off, c)
                    i += len(carriers)
                i += 1


def _get_nc(feat_dtype="float32"):
    if feat_dtype not in _BUILT:
        _BUILT[feat_dtype] = _build(feat_dtype)
    return _BUILT[feat_dtype]


def _permute_pix(arr, nsb):
    """[nsb*2048, f] logical pixel-major -> [128, nsb*K*f] on-chip layout."""
    f = arr.shape[-1]
    return np.ascontiguousarray(
        arr.reshape(nsb, PART, K, f).transpose(1, 0, 2, 3).reshape(PART, -1)
    )


def ref_core_numpy(features, aux_i, aux_f, dirs):
    """Numpy oracle for one core's flat-pixel program (for validation).
    All arrays in logical pixel-major layout: aux_i [npix,4], aux_f [npix,6],
    dirs [NPIX_F,3]."""
    x, y, z = dirs[:, 0], dirs[:, 1], dirs[:, 2]
    bas = np.stack([
        np.full_like(x, SH_C0), -SH_C1 * y, SH_C1 * z, -SH_C1 * x,
        SH_C2[0] * x * y, SH_C2[1] * y * z,
        SH_C2[2] * (2 * z * z - x * x - y * y),
        SH_C2[3] * x * z, SH_C2[4] * (x * x - y * y),
    ], axis=-1)                                        # [NPIX_F, 9]
    npix = aux_i.shape[0]
    reps = -(-npix // bas.shape[0])
    bas = np.concatenate([bas] * reps, axis=0)[:npix]
    g = features[aux_i]                                # [npix, 4, 72]
    d, b0, b1, b2, m1, m2 = (aux_f[:, i] for i in range(6))
    wm1 = m1 * (1.0 - d / R2)
    f1 = (g[:, 0].reshape(npix, 8, 9) * bas[:, None, :]).sum(-1)
    col = (b0[:, None] * g[:, 1] + b1[:, None] * g[:, 2] + b2[:, None] * g[:, 3])
    f2 = (col.reshape(npix, 8, 9) * bas[:, None, :]).sum(-1)
    out = np.concatenate([f1 * wm1[:, None], f2 * m2[:, None]], axis=-1)
    mask = np.maximum(m1, m2)
    return out.astype(np.float32), mask.astype(np.float32)


def _prep_core_inputs(features_np, frag1_dists, frag2_bary, flipped_dirs,
                      faces_np, frag1_idx, frag2_pix_to_face, core):
    hs = slice(core * HROWS, (core + 1) * HROWS)
    idx1 = frag1_idx[:, hs, :, 0]
    p2f = frag2_pix_to_face[:, hs, :, 0]
    m1 = idx1 >= 0
    m2 = p2f >= 0
    c1 = np.maximum(idx1, 0).astype(np.int32)
    tri = faces_np[np.maximum(p2f, 0)].astype(np.int32)  # [2,96,768,3]
    aux_i = _permute_pix(
        np.concatenate([c1[..., None], tri], axis=-1).reshape(-1, 4), NSB
    )
    d = frag1_dists[:, hs, :, 0]
    bary = frag2_bary[:, hs, :, 0, :]                    # [2,96,768,3]
    aux_f = np.ascontiguousarray(
        np.concatenate(
            [d[..., None], bary,
             m1[..., None].astype(np.float32), m2[..., None].astype(np.float32)],
            axis=-1,
        ).reshape(-1, 6).astype(np.float32)
    )
    dirs_core = _permute_pix(
        flipped_dirs[hs].reshape(-1, 3).astype(np.float32), NSB_F
    )
    return {"aux_i": aux_i, "aux_f": aux_f, "dirs": dirs_core,
            "features": features_np}


def run(inputs, trace=False, tmpdir=None):
    """Run the SPMD kernel; returns (outputs_tuple, BassKernelResults)."""
    from concourse.bass_utils import run_bass_kernel_spmd

    features = np.ascontiguousarray(np.asarray(inputs["features"], dtype=np.float32))
    frag1_dists = np.asarray(inputs["frag1_dists"], dtype=np.float32)
    frag2_bary = np.asarray(inputs["frag2_bary"], dtype=np.float32)
    ray_dirs = np.asarray(inputs["ray_dirs"], dtype=np.float32)
    faces = np.asarray(inputs["faces"]).astype(np.int64)
    frag1_idx = np.asarray(inputs["frag1_idx"]).astype(np.int64)
    frag2_p2f = np.asarray(inputs["frag2_pix_to_face"]).astype(np.int64)

    flipped = np.flip(ray_dirs, axis=(1, 2)).transpose(1, 2, 0)  # [H,W,3]

    in_maps = [
        _prep_core_inputs(features, frag1_dists, frag2_bary, flipped,
                          faces, frag1_idx, frag2_p2f, core)
        for core in range(NCORES)
    ]
    nc = _get_nc("float32")
    res = run_bass_kernel_spmd(nc, in_maps, list(range(NCORES)), trace=trace,
                               tmpdir=tmpdir)

    of = np.stack([m["out_feat"] for m in res.results])   # [8, NPIX, 16]
    of = of.reshape(NCORES, 2, HROWS, W, 16).transpose(1, 0, 2, 3, 4)
    of = np.ascontiguousarray(of.reshape(2, H, W, 16))
    mk = np.stack([m["out_mask"] for m in res.results])
    mk = mk.reshape(NCORES, 2, HROWS, W).transpose(1, 0, 2, 3)
    mk = (mk.reshape(2, H, W, 1) > 0.5)
    out = (of[0:1], of[1:2], mk[0:1], mk[1:2])
    return out, res


def kernel(**inputs):
    out, _ = run(inputs, trace=False)
    return out
